# revision 7
# baseline (speedup 1.0000x reference)
"""Trainium2 Bass kernel for nn_Attention_50354196578449 (sparse_attention).

Reference computation (per batch b of B=64, N=512, MD=QD=AD=1024):
    tq      = query @ Ws                                   # (B, AD)
    h       = tanh(memory_values @ Wh + tq[:, None, :])    # (B, N, AD)
    logits  = squeeze(h @ v)                               # (B, N)
    weights = masked softmax(logits)                       # (B, N)
    context = einsum("bn,bnd->bd", weights, memory_values) # (B, MD)

Strategy: data-parallel over batch across 8 NeuronCores (8 batches/core).
Per core, everything is fused on-chip:
  - mv is DMA'd once per batch in natural layout [n, md], transposed to
    [md, n] tiles on the PE (f32r transposes), kept in SBUF.
  - h^T tiles [ad=128, n=512] are produced by f32r matmuls
    (Wh chunk stationary, mv^T chunk moving) accumulated over md in PSUM,
    then ACT applies tanh with the per-partition bias tq^T[:, b].
  - logits accumulate in PSUM via v-chunk (M=1) f32r matmuls over ad.
  - masked softmax runs on partition 0 (DVE/ACT small ops).
  - context = w^T-chunk (M=1) f32r matmuls against the natural-layout mv
    tiles, contracting over n.
"""

import sys

sys.path.insert(0, "/opt/trn_rl_repo")

from contextlib import ExitStack

import numpy as np

N_CORES = 8
B = 64
B_LOC = B // N_CORES  # 8 batches per core
N = 512
MD = 1024
QD = 1024
AD = 1024
P = 128
NMD = MD // P  # 8 md chunks
NAD = AD // P  # 8 ad chunks
NQD = QD // P  # 8 qd chunks
NNT = N // P   # 4 n chunks

_CACHE = {}


def _build_nc():
    import concourse.bass as bass  # noqa: F401
    import concourse.tile as tile
    from concourse import bacc, mybir
    from concourse.masks import make_identity

    F32 = mybir.dt.float32
    F32R = mybir.dt.float32r
    I32 = mybir.dt.int32
    AF = mybir.ActivationFunctionType
    OP = mybir.AluOpType
    AX = mybir.AxisListType

    nc = bacc.Bacc("TRN2", target_bir_lowering=False)

    mv_d = nc.dram_tensor("memory_values", (B_LOC, N, MD), F32,
                          kind="ExternalInput")
    mask_d = nc.dram_tensor("mask", (B_LOC, N), I32, kind="ExternalInput")
    query_d = nc.dram_tensor("query", (B_LOC, QD), F32, kind="ExternalInput")
    Wh_d = nc.dram_tensor("Wh", (MD, AD), F32, kind="ExternalInput")
    Ws_d = nc.dram_tensor("Ws", (QD, AD), F32, kind="ExternalInput")
    v_d = nc.dram_tensor("v", (AD, 1), F32, kind="ExternalInput")
    ctx_d = nc.dram_tensor("context", (B_LOC, MD), F32, kind="ExternalOutput")

    with tile.TileContext(nc) as tc, ExitStack() as ctx:
        const = ctx.enter_context(tc.tile_pool(name="const", bufs=1))
        nat_pool = ctx.enter_context(tc.tile_pool(name="nat", bufs=4))
        mvT_pool = ctx.enter_context(tc.tile_pool(name="mvT", bufs=10))
        hT_pool = ctx.enter_context(tc.tile_pool(name="hT", bufs=3))
        small = ctx.enter_context(tc.tile_pool(name="small", bufs=2))
        out_pool = ctx.enter_context(tc.tile_pool(name="outp", bufs=2))
        psum_h = ctx.enter_context(
            tc.tile_pool(name="psum_h", bufs=2, space="PSUM"))
        psum_tr = ctx.enter_context(
            tc.tile_pool(name="psum_tr", bufs=2, space="PSUM"))
        psum_sm = ctx.enter_context(
            tc.tile_pool(name="psum_sm", bufs=4, space="PSUM"))

        # ---- constants ----------------------------------------------------
        ident_f = const.tile([P, P], F32)
        make_identity(nc, ident_f[:])
        ident_r = const.tile([P, P], F32R)
        nc.vector.tensor_copy(ident_r[:], ident_f[:])

        # Wh as [p, mdc, ad] so lhsT chunks are [:, mdc, adc*128:+128]
        Wh_sb = const.tile([P, NMD, AD], F32R)
        nc.sync.dma_start(
            Wh_sb[:], Wh_d[:].rearrange("(c p) a -> p c a", p=P).bitcast(F32R))

        # v padded to 8B stride: [p, adc, 2] with value in slot 0
        v_sb = const.tile([P, NAD, 2], F32R)
        nc.sync.dma_start(
            v_sb[:, :, 0:1],
            v_d[:].rearrange("(c p) x -> p c x", p=P).bitcast(F32R))

        # query^T [p, qc, b]: natural load + PE transposes
        q_sb = const.tile([B_LOC, QD], F32R)
        nc.sync.dma_start(q_sb[:], query_d[:].bitcast(F32R))
        qT_sb = const.tile([P, NQD, B_LOC], F32R)
        for c in range(NQD):
            ps_q = psum_sm.tile([P, B_LOC], F32R, tag="sm")
            nc.tensor.transpose(ps_q[:], q_sb[:, c * P:(c + 1) * P],
                                ident_r[:B_LOC, :B_LOC])
            nc.vector.tensor_copy(qT_sb[:, c, :], ps_q[:])

        # ---- mask precompute (partition 0) --------------------------------
        mask_i = nat_pool.tile([1, B_LOC, N], I32, tag="nat")
        nc.sync.dma_start(mask_i[:], mask_d[:])
        maskf = nat_pool.tile([1, B_LOC, N], F32, tag="nat")
        nc.vector.tensor_copy(maskf[:], mask_i[:])
        # mx[b] = max over n (1 if any cell valid, else 0)
        mx = const.tile([1, B_LOC, 1], F32)
        nc.vector.reduce_max(mx[:], maskf[:], axis=AX.X)
        # suppress = maskf * 1e30 - 1e30  (0 where valid, -1e30 where masked)
        suppress = const.tile([1, B_LOC, N], F32)
        nc.vector.tensor_scalar(suppress[:], maskf[:], 1e30, -1e30,
                                op0=OP.mult, op1=OP.add)

        # ---- tq^T = (query @ Ws)^T as [p(ad), adc, b] ---------------------
        Ws_h = []
        for h in range(2):
            t = nat_pool.tile([P, NQD // 2, AD], F32R, tag="nat")
            nc.sync.dma_start(
                t[:],
                Ws_d[:].rearrange("(c p) a -> p c a", p=P)[
                    :, h * (NQD // 2):(h + 1) * (NQD // 2), :].bitcast(F32R))
            Ws_h.append(t)
        tqT_sb = const.tile([P, NAD, B_LOC], F32)
        for adc in range(NAD):
            ps_tq = psum_sm.tile([P, B_LOC], F32, tag="sm")
            for qc in range(NQD):
                nc.tensor.matmul(
                    ps_tq[:],
                    Ws_h[qc // 4][:, qc % 4, adc * P:(adc + 1) * P],
                    qT_sb[:, qc, :],
                    start=(qc == 0), stop=(qc == NQD - 1))
            nc.vector.tensor_copy(tqT_sb[:, adc, :], ps_tq[:])

        # persistent softmax result tile
        w_sb = const.tile([1, B_LOC, N], F32)

        # ---- main loop over local batches ---------------------------------
        nats = [None] * B_LOC
        wTs = [None] * B_LOC

        def emit_wT_D_out(b):
            """Transpose w(b), then context matmuls against natural mv(b)."""
            ps_wT = psum_sm.tile([P, NNT], F32, tag="sm")
            for t in range(NNT):
                nc.tensor.matmul(ps_wT[:, t:t + 1],
                                 w_sb[0:1, b, t * P:(t + 1) * P],
                                 ident_f[0:1, 0:1], is_transpose=True)
            wT = small.tile([P, NNT, 2], F32R, tag="wT")
            nc.vector.tensor_copy(wT[:, :, 0:1], ps_wT[:].unsqueeze(-1))
            wTs[b] = wT
            out_sb = out_pool.tile([1, MD], F32)
            for md2 in range(2):
                ps_c = psum_sm.tile([1, MD // 2], F32, tag="sm")
                for t in range(NNT):
                    nc.tensor.matmul(
                        ps_c[:], wT[:, t, 0:1],
                        nats[b][:, t, md2 * 512:(md2 + 1) * 512],
                        start=(t == 0), stop=(t == NNT - 1),
                        skip_group_check=True)
                nc.vector.tensor_copy(out_sb[0:1, md2 * 512:(md2 + 1) * 512],
                                      ps_c[:])
            nc.sync.dma_start(ctx_d[b:b + 1, :], out_sb[:])

        for b in range(B_LOC):
            # natural-layout load [p(n), nt, md]
            nat = nat_pool.tile([P, NNT, MD], F32R, tag="nat")
            nc.sync.dma_start(
                nat[:],
                mv_d[b].rearrange("(t p) m -> p t m", p=P).bitcast(F32R))
            nats[b] = nat

            # transpose to mv^T tiles [p(md), n] per md chunk
            mvT = []
            for mdc in range(NMD):
                ps_t = psum_tr.tile([P, N], F32R)
                for t in range(NNT):
                    nc.tensor.transpose(
                        ps_t[:, t * P:(t + 1) * P],
                        nat[:, t, mdc * P:(mdc + 1) * P], ident_r[:])
                mt = mvT_pool.tile([P, N], F32R)
                if mdc % 2 == 0:
                    nc.vector.tensor_copy(mt[:], ps_t[:])
                else:
                    nc.scalar.copy(mt[:], ps_t[:])
                mvT.append(mt)

            # h^T tiles + logits accumulation
            ps_log = psum_sm.tile([1, N], F32, tag="sm")
            hTs = []
            for adc in range(NAD):
                ps_h = psum_h.tile([P, N], F32)
                for mdc in range(NMD):
                    nc.tensor.matmul(
                        ps_h[:], Wh_sb[:, mdc, adc * P:(adc + 1) * P],
                        mvT[mdc], start=(mdc == 0), stop=(mdc == NMD - 1))
                hT = hT_pool.tile([P, N], F32R)
                nc.scalar.activation(hT[:], ps_h[:], AF.Tanh,
                                     bias=tqT_sb[:, adc, b:b + 1])
                hTs.append(hT)
                if adc >= 1:
                    nc.tensor.matmul(ps_log[:], v_sb[:, adc - 1, 0:1],
                                     hTs[adc - 1], start=(adc - 1 == 0),
                                     stop=False, skip_group_check=True)

            # fill the tanh(7) latency with last batch's context stage
            if b > 0:
                emit_wT_D_out(b - 1)

            nc.tensor.matmul(ps_log[:], v_sb[:, NAD - 1, 0:1], hTs[NAD - 1],
                             start=False, stop=True, skip_group_check=True)

            # masked softmax on partition 0
            ml = small.tile([1, N], F32, tag="ml")
            nc.vector.scalar_tensor_tensor(
                ml[:], in0=suppress[0:1, b, :], scalar=mx[0:1, b, :],
                in1=ps_log[:], op0=OP.mult, op1=OP.add)
            rmx = small.tile([1, 1], F32, tag="rmx")
            nc.vector.reduce_max(rmx[:], ml[:], axis=AX.X)
            nrmx = small.tile([1, 1], F32, tag="nrmx")
            nc.vector.tensor_scalar(nrmx[:], rmx[:], -1.0, None, op0=OP.mult)
            et = small.tile([1, N], F32, tag="et")
            zs = small.tile([1, 1], F32, tag="zs")
            nc.scalar.activation(et[:], ml[:], AF.Exp, bias=nrmx[:],
                                 accum_out=zs[:])
            rz = small.tile([1, 1], F32, tag="rz")
            nc.vector.reciprocal(rz[:], zs[:])
            nc.vector.tensor_scalar(w_sb[0:1, b, :], et[:], rz[:],
                                    mx[0:1, b, :], op0=OP.mult, op1=OP.mult)

        emit_wT_D_out(B_LOC - 1)

    nc.compile()
    return nc


def _get_nc():
    if "nc" not in _CACHE:
        _CACHE["nc"] = _build_nc()
    return _CACHE["nc"]


def kernel(memory_values, mask, query, Wh, Ws, v):
    from concourse.bass_utils import run_bass_kernel_spmd

    nc = _get_nc()
    memory_values = np.ascontiguousarray(memory_values, dtype=np.float32)
    mask = np.ascontiguousarray(mask, dtype=np.int32)
    query = np.ascontiguousarray(query, dtype=np.float32)
    Wh = np.ascontiguousarray(Wh, dtype=np.float32)
    Ws = np.ascontiguousarray(Ws, dtype=np.float32)
    v = np.ascontiguousarray(v, dtype=np.float32)

    in_maps = []
    for c in range(N_CORES):
        s = slice(c * B_LOC, (c + 1) * B_LOC)
        in_maps.append({
            "memory_values": memory_values[s],
            "mask": mask[s],
            "query": query[s],
            "Wh": Wh,
            "Ws": Ws,
            "v": v,
        })
    res = run_bass_kernel_spmd(nc, in_maps, core_ids=list(range(N_CORES)))
    out = np.concatenate([res.results[c]["context"] for c in range(N_CORES)],
                         axis=0)
    return out.astype(np.float32)


# revision 8
# speedup vs baseline: 1.1931x; 1.1931x over previous
"""Trainium2 Bass kernel for nn_Attention_50354196578449 (sparse_attention).

Reference computation (per batch b of B=64, N=512, MD=QD=AD=1024):
    tq      = query @ Ws                                   # (B, AD)
    h       = tanh(memory_values @ Wh + tq[:, None, :])    # (B, N, AD)
    logits  = squeeze(h @ v)                               # (B, N)
    weights = masked softmax(logits)                       # (B, N)
    context = einsum("bn,bnd->bd", weights, memory_values) # (B, MD)

Strategy: data-parallel over batch across 8 NeuronCores (8 batches/core).
Per core, fully fused on-chip:
  - mv is loaded once per batch as fp16 (gpsimd cast-DMA) in natural
    layout [n, md]; mv^T tiles come from xbar DMA-transposes
    (one [128,1024] -> [128,8,128] SBUF->SBUF transpose per n-chunk).
  - h^T tiles [ad=128, n=512] via fp16 matmuls (Wh chunk stationary,
    mv^T chunk moving) accumulated over md in PSUM; ACT applies tanh
    with the per-partition bias tq^T[:, b].
  - logits accumulate in PSUM via v-chunk (M=1) fp16 matmuls over ad.
  - masked softmax on partition 0 (DVE/ACT small ops, fp32).
  - context = w^T-chunk (M=1) fp16 matmuls against the natural-layout
    fp16 mv tiles, contracting over n.
  - a dummy-matmul warmup stream keeps the PE HAM clock-gate open while
    the prologue DMAs land.
"""

import sys

sys.path.insert(0, "/opt/trn_rl_repo")

from contextlib import ExitStack

import numpy as np

N_CORES = 8
B = 64
B_LOC = B // N_CORES  # 8 batches per core
N = 512
MD = 1024
QD = 1024
AD = 1024
P = 128
NMD = MD // P  # 8 md chunks
NAD = AD // P  # 8 ad chunks
NQD = QD // P  # 8 qd chunks
NNT = N // P   # 4 n chunks
WARMUP_MMS = 150

_CACHE = {}


def _build_nc():
    import concourse.bass as bass  # noqa: F401
    import concourse.tile as tile
    from concourse import bacc, mybir
    from concourse.masks import make_identity

    F32 = mybir.dt.float32
    F16 = mybir.dt.float16
    I32 = mybir.dt.int32
    AF = mybir.ActivationFunctionType
    OP = mybir.AluOpType
    AX = mybir.AxisListType

    nc = bacc.Bacc("TRN2", target_bir_lowering=False)

    mv_d = nc.dram_tensor("memory_values", (B_LOC, N, MD), F32,
                          kind="ExternalInput")
    mask_d = nc.dram_tensor("mask", (B_LOC, N), I32, kind="ExternalInput")
    query_d = nc.dram_tensor("query", (B_LOC, QD), F32, kind="ExternalInput")
    Wh_d = nc.dram_tensor("Wh", (MD, AD), F32, kind="ExternalInput")
    Ws_d = nc.dram_tensor("Ws", (QD, AD), F32, kind="ExternalInput")
    v_d = nc.dram_tensor("v", (AD, 1), F32, kind="ExternalInput")
    ctx_d = nc.dram_tensor("context", (B_LOC, MD), F32, kind="ExternalOutput")

    with tile.TileContext(nc) as tc, ExitStack() as ctx:
        const = ctx.enter_context(tc.tile_pool(name="const", bufs=1))
        nath_pool = ctx.enter_context(tc.tile_pool(name="nath", bufs=3))
        mvT_pool = ctx.enter_context(tc.tile_pool(name="mvT", bufs=2))
        hT_pool = ctx.enter_context(tc.tile_pool(name="hT", bufs=3))
        small = ctx.enter_context(tc.tile_pool(name="small", bufs=2))
        out_pool = ctx.enter_context(tc.tile_pool(name="outp", bufs=2))
        misc_pool = ctx.enter_context(tc.tile_pool(name="misc", bufs=2))
        psum_h = ctx.enter_context(
            tc.tile_pool(name="psum_h", bufs=2, space="PSUM"))
        psum_tr = ctx.enter_context(
            tc.tile_pool(name="psum_tr", bufs=2, space="PSUM"))
        psum_sm = ctx.enter_context(
            tc.tile_pool(name="psum_sm", bufs=4, space="PSUM"))

        # ---- identities + PE warmup (keeps HAM at full clock while the
        # ---- prologue DMAs stream in) --------------------------------------
        ident_f = const.tile([P, P], F32)
        make_identity(nc, ident_f[:])
        ident_h = const.tile([P, P], F16)
        nc.vector.tensor_copy(ident_h[:], ident_f[:])

        warm_ps = psum_tr.tile([P, P], F32, tag="tr")
        for _ in range(WARMUP_MMS):
            nc.tensor.matmul(warm_ps[:], ident_h[:], ident_h[:],
                             start=True, stop=True)
        warm_sink = const.tile([P, 1], F32)
        nc.vector.tensor_copy(warm_sink[:], warm_ps[:, 0:1])

        # ---- fp16 casts of the weights/inputs (gpsimd cast-DMA) -----------
        Wh_sb = const.tile([P, NMD, AD], F16)
        nc.gpsimd.dma_start(
            Wh_sb[:], Wh_d[:].rearrange("(c p) a -> p c a", p=P))
        Ws_sb = const.tile([P, NQD, AD], F16)
        nc.gpsimd.dma_start(
            Ws_sb[:], Ws_d[:].rearrange("(c p) a -> p c a", p=P))
        v_sb = const.tile([P, NAD, 2], F16)
        nc.gpsimd.dma_start(
            v_sb[:, :, 0:1], v_d[:].rearrange("(c p) x -> p c x", p=P))
        q_sb = const.tile([B_LOC, QD], F16)
        nc.gpsimd.dma_start(q_sb[:], query_d[:])

        # ---- mask precompute (partition 0) --------------------------------
        mask_i = misc_pool.tile([1, B_LOC, N], I32, tag="mask")
        nc.sync.dma_start(mask_i[:], mask_d[:])
        maskf = misc_pool.tile([1, B_LOC, N], F32, tag="mask")
        nc.vector.tensor_copy(maskf[:], mask_i[:])
        mx = const.tile([1, B_LOC, 1], F32)
        nc.vector.reduce_max(mx[:], maskf[:], axis=AX.X)
        suppress = const.tile([1, B_LOC, N], F32)
        nc.vector.tensor_scalar(suppress[:], maskf[:], 1e30, -1e30,
                                op0=OP.mult, op1=OP.add)

        # ---- query^T + tq^T = (query @ Ws)^T as [p(ad), adc, b] -----------
        qT_sb = const.tile([P, NQD, B_LOC], F16)
        for c in range(NQD):
            ps_q = psum_tr.tile([P, B_LOC], F16, tag="tr")
            nc.tensor.transpose(ps_q[:], q_sb[:, c * P:(c + 1) * P],
                                ident_h[:B_LOC, :B_LOC])
            nc.vector.tensor_copy(qT_sb[:, c, :], ps_q[:])
        tqT_sb = const.tile([P, NAD, B_LOC], F32)
        for adc in range(NAD):
            ps_tq = psum_sm.tile([P, B_LOC], F32, tag="sm")
            for qc in range(NQD):
                nc.tensor.matmul(
                    ps_tq[:], Ws_sb[:, qc, adc * P:(adc + 1) * P],
                    qT_sb[:, qc, :], start=(qc == 0), stop=(qc == NQD - 1))
            nc.vector.tensor_copy(tqT_sb[:, adc, :], ps_tq[:])

        # persistent softmax result tile
        w_sb = const.tile([1, B_LOC, N], F32)

        # ---- main loop over local batches ---------------------------------
        naths = [None] * B_LOC
        mvTs = [None] * B_LOC

        def emit_loads(b):
            """fp16 natural-layout load + xbar transposes for batch b."""
            nath = nath_pool.tile([P, NNT, MD], F16, tag="nath")
            nc.gpsimd.dma_start(
                nath[:], mv_d[b].rearrange("(t p) m -> p t m", p=P))
            naths[b] = nath
            mvT = mvT_pool.tile([P, NMD, N], F16, tag="mvT")
            for t in range(NNT):
                nc.sync.dma_start_transpose(
                    mvT[:, :, t * P:(t + 1) * P], nath[:, t, :])
            mvTs[b] = mvT

        def emit_wT_D_out(b):
            """Transpose w(b), then context matmuls against natural mv(b)."""
            ps_wT = psum_sm.tile([P, NNT], F32, tag="sm")
            for t in range(NNT):
                nc.tensor.matmul(ps_wT[:, t:t + 1],
                                 w_sb[0:1, b, t * P:(t + 1) * P],
                                 ident_f[0:1, 0:1], is_transpose=True,
                                 skip_group_check=True)
            wT = small.tile([P, NNT, 2], F16, tag="wT")
            nc.vector.tensor_copy(wT[:, :, 0:1], ps_wT[:].unsqueeze(-1))
            out_sb = out_pool.tile([1, MD], F32)
            for md2 in range(2):
                ps_c = psum_sm.tile([1, MD // 2], F32, tag="sm")
                for t in range(NNT):
                    nc.tensor.matmul(
                        ps_c[:], wT[:, t, 0:1],
                        naths[b][:, t, md2 * 512:(md2 + 1) * 512],
                        start=(t == 0), stop=(t == NNT - 1),
                        skip_group_check=True)
                nc.vector.tensor_copy(out_sb[0:1, md2 * 512:(md2 + 1) * 512],
                                      ps_c[:])
            nc.sync.dma_start(ctx_d[b:b + 1, :], out_sb[:])

        emit_loads(0)

        for b in range(B_LOC):
            if b + 1 < B_LOC:
                emit_loads(b + 1)

            mvT = mvTs[b]
            ps_log = psum_sm.tile([1, N], F32, tag="sm")
            hTs = []
            for adc in range(NAD):
                ps_h = psum_h.tile([P, N], F32)
                for mdc in range(NMD):
                    nc.tensor.matmul(
                        ps_h[:], Wh_sb[:, mdc, adc * P:(adc + 1) * P],
                        mvT[:, mdc, :], start=(mdc == 0),
                        stop=(mdc == NMD - 1))
                hT = hT_pool.tile([P, N], F16)
                nc.scalar.activation(hT[:], ps_h[:], AF.Tanh,
                                     bias=tqT_sb[:, adc, b:b + 1])
                hTs.append(hT)
                if adc >= 1:
                    nc.tensor.matmul(ps_log[:], v_sb[:, adc - 1, 0:1],
                                     hTs[adc - 1], start=(adc - 1 == 0),
                                     stop=False, skip_group_check=True)

            # fill the tanh(7) latency with last batch's context stage
            if b > 0:
                emit_wT_D_out(b - 1)

            nc.tensor.matmul(ps_log[:], v_sb[:, NAD - 1, 0:1], hTs[NAD - 1],
                             start=False, stop=True, skip_group_check=True)

            # masked softmax on partition 0
            ml = small.tile([1, N], F32, tag="ml")
            nc.vector.scalar_tensor_tensor(
                ml[:], in0=suppress[0:1, b, :], scalar=mx[0:1, b, :],
                in1=ps_log[:], op0=OP.mult, op1=OP.add)
            rmx = small.tile([1, 1], F32, tag="rmx")
            nc.vector.reduce_max(rmx[:], ml[:], axis=AX.X)
            nrmx = small.tile([1, 1], F32, tag="nrmx")
            nc.vector.tensor_scalar(nrmx[:], rmx[:], -1.0, None, op0=OP.mult)
            et = small.tile([1, N], F32, tag="et")
            zs = small.tile([1, 1], F32, tag="zs")
            nc.scalar.activation(et[:], ml[:], AF.Exp, bias=nrmx[:],
                                 accum_out=zs[:])
            rz = small.tile([1, 1], F32, tag="rz")
            nc.vector.reciprocal(rz[:], zs[:])
            nc.vector.tensor_scalar(w_sb[0:1, b, :], et[:], rz[:],
                                    mx[0:1, b, :], op0=OP.mult, op1=OP.mult)

        emit_wT_D_out(B_LOC - 1)

    nc.compile()
    return nc


def _get_nc():
    if "nc" not in _CACHE:
        _CACHE["nc"] = _build_nc()
    return _CACHE["nc"]


def kernel(memory_values, mask, query, Wh, Ws, v):
    from concourse.bass_utils import run_bass_kernel_spmd

    nc = _get_nc()
    memory_values = np.ascontiguousarray(memory_values, dtype=np.float32)
    mask = np.ascontiguousarray(mask, dtype=np.int32)
    query = np.ascontiguousarray(query, dtype=np.float32)
    Wh = np.ascontiguousarray(Wh, dtype=np.float32)
    Ws = np.ascontiguousarray(Ws, dtype=np.float32)
    v = np.ascontiguousarray(v, dtype=np.float32)

    in_maps = []
    for c in range(N_CORES):
        s = slice(c * B_LOC, (c + 1) * B_LOC)
        in_maps.append({
            "memory_values": memory_values[s],
            "mask": mask[s],
            "query": query[s],
            "Wh": Wh,
            "Ws": Ws,
            "v": v,
        })
    res = run_bass_kernel_spmd(nc, in_maps, core_ids=list(range(N_CORES)))
    out = np.concatenate([res.results[c]["context"] for c in range(N_CORES)],
                         axis=0)
    return out.astype(np.float32)


# revision 12
# speedup vs baseline: 1.2021x; 1.0076x over previous
"""Trainium2 Bass kernel for nn_Attention_50354196578449 (sparse_attention).

Reference computation (per batch b of B=64, N=512, MD=QD=AD=1024):
    tq      = query @ Ws                                   # (B, AD)
    h       = tanh(memory_values @ Wh + tq[:, None, :])    # (B, N, AD)
    logits  = squeeze(h @ v)                               # (B, N)
    weights = masked softmax(logits)                       # (B, N)
    context = einsum("bn,bnd->bd", weights, memory_values) # (B, MD)

Strategy: data-parallel over batch across 8 NeuronCores (8 batches/core).
Per core, fully fused on-chip:
  - mv is loaded once per batch as fp16 (gpsimd cast-DMA) in natural
    layout [n, md]; mv^T tiles come from xbar DMA-transposes
    (one [128,1024] -> [128,8,128] SBUF->SBUF transpose per n-chunk).
  - h^T tiles [ad=128, n=512] via fp16 matmuls (Wh chunk stationary,
    mv^T chunk moving) accumulated over md in PSUM; ACT applies tanh
    with the per-partition bias tq^T[:, b].
  - logits accumulate in PSUM via v-chunk (M=1) fp16 matmuls over ad.
  - masked softmax on partition 0 (DVE/ACT small ops, fp32).
  - context = w^T-chunk (M=1) fp16 matmuls against the natural-layout
    fp16 mv tiles, contracting over n.
  - a dummy-matmul warmup stream keeps the PE HAM clock-gate open while
    the prologue DMAs land.
"""

import sys

sys.path.insert(0, "/opt/trn_rl_repo")

from contextlib import ExitStack

import numpy as np

N_CORES = 8
B = 64
B_LOC = B // N_CORES  # 8 batches per core
N = 512
MD = 1024
QD = 1024
AD = 1024
P = 128
NMD = MD // P  # 8 md chunks
NAD = AD // P  # 8 ad chunks
NQD = QD // P  # 8 qd chunks
NNT = N // P   # 4 n chunks
WARMUP_MMS = 150

_CACHE = {}


def _build_nc():
    import concourse.bass as bass  # noqa: F401
    import concourse.tile as tile
    from concourse import bacc, mybir
    from concourse.masks import make_identity

    F32 = mybir.dt.float32
    F16 = mybir.dt.float16
    I32 = mybir.dt.int32
    AF = mybir.ActivationFunctionType
    OP = mybir.AluOpType
    AX = mybir.AxisListType

    nc = bacc.Bacc("TRN2", target_bir_lowering=False)

    mv_d = nc.dram_tensor("memory_values", (B_LOC, N, MD), F32,
                          kind="ExternalInput")
    mask_d = nc.dram_tensor("mask", (B_LOC, N), I32, kind="ExternalInput")
    query_d = nc.dram_tensor("query", (B_LOC, QD), F32, kind="ExternalInput")
    Wh_d = nc.dram_tensor("Wh", (MD, AD), F32, kind="ExternalInput")
    Ws_d = nc.dram_tensor("Ws", (QD, AD), F32, kind="ExternalInput")
    v_d = nc.dram_tensor("v", (AD, 1), F32, kind="ExternalInput")
    ctx_d = nc.dram_tensor("context", (B_LOC, MD), F32, kind="ExternalOutput")

    with tile.TileContext(nc) as tc, ExitStack() as ctx:
        const = ctx.enter_context(tc.tile_pool(name="const", bufs=1))
        nath_pool = ctx.enter_context(tc.tile_pool(name="nath", bufs=3))
        mvT_pool = ctx.enter_context(tc.tile_pool(name="mvT", bufs=2))
        hT_pool = ctx.enter_context(tc.tile_pool(name="hT", bufs=3))
        small = ctx.enter_context(tc.tile_pool(name="small", bufs=2))
        out_pool = ctx.enter_context(tc.tile_pool(name="outp", bufs=2))
        misc_pool = ctx.enter_context(tc.tile_pool(name="misc", bufs=2))
        stage = ctx.enter_context(tc.tile_pool(name="stage", bufs=1))
        psum_h = ctx.enter_context(
            tc.tile_pool(name="psum_h", bufs=3, space="PSUM"))
        psum_tr = ctx.enter_context(
            tc.tile_pool(name="psum_tr", bufs=1, space="PSUM"))
        psum_sm = ctx.enter_context(
            tc.tile_pool(name="psum_sm", bufs=4, space="PSUM"))

        # ---- identities + PE warmup (keeps HAM at full clock while the
        # ---- prologue DMAs stream in) --------------------------------------
        ident_f = const.tile([P, P], F32)
        make_identity(nc, ident_f[:])
        ident_h = const.tile([P, P], F16)
        nc.vector.tensor_copy(ident_h[:], ident_f[:])

        warm_ps = psum_tr.tile([P, P], F32, tag="tr")
        for _ in range(WARMUP_MMS):
            nc.tensor.matmul(warm_ps[:], ident_h[:], ident_h[:],
                             start=True, stop=True)
        warm_sink = const.tile([P, 1], F32)
        nc.vector.tensor_copy(warm_sink[:], warm_ps[:, 0:1])

        # ---- loads, in criticality order ----------------------------------
        # Small HWDGE loads first (query / v / mask), then the first batch
        # plus Wh on the SWDGE cast path, while Ws streams in fp32 on HWDGE
        # and is cast on-chip.
        qf_sb = misc_pool.tile([B_LOC, QD], F32, tag="qf")
        nc.sync.dma_start(qf_sb[:], query_d[:])
        q_sb = const.tile([B_LOC, QD], F16)
        nc.vector.tensor_copy(q_sb[:], qf_sb[:])
        vf_sb = misc_pool.tile([P, NAD], F32, tag="vf")
        nc.sync.dma_start(vf_sb[:], v_d[:].rearrange("(c p) x -> p (c x)", p=P))
        v_sb = const.tile([P, NAD, 2], F16)
        nc.vector.tensor_copy(v_sb[:, :, 0:1], vf_sb[:].unsqueeze(-1))
        mask_i = misc_pool.tile([1, B_LOC, N], I32, tag="mask")
        nc.sync.dma_start(mask_i[:], mask_d[:])

        # first batch + Wh on the SWDGE cast queue
        naths = [None] * B_LOC
        mvTs = [None] * B_LOC

        def emit_loads(b):
            """fp16 natural-layout load + xbar transposes for batch b."""
            nath = nath_pool.tile([P, NNT, MD], F16, tag="nath")
            nc.gpsimd.dma_start(
                nath[:], mv_d[b].rearrange("(t p) m -> p t m", p=P))
            naths[b] = nath
            mvT = mvT_pool.tile([P, NMD, N], F16, tag="mvT")
            for t in range(NNT):
                nc.sync.dma_start_transpose(
                    mvT[:, :, t * P:(t + 1) * P], nath[:, t, :])
            mvTs[b] = mvT

        emit_loads(0)
        Wh_sb = const.tile([P, NMD, AD], F16)
        nc.gpsimd.dma_start(
            Wh_sb[:], Wh_d[:].rearrange("(c p) a -> p c a", p=P))

        # Ws: fp32 HWDGE load + on-chip cast (keeps it off the SWDGE queue)
        Wsf = stage.tile([P, NQD, AD], F32)
        nc.sync.dma_start(Wsf[:], Ws_d[:].rearrange("(c p) a -> p c a", p=P))
        Ws_sb = const.tile([P, NQD, AD], F16)
        nc.vector.tensor_copy(Ws_sb[:, 0:4, :], Wsf[:, 0:4, :])
        nc.scalar.copy(Ws_sb[:, 4:8, :], Wsf[:, 4:8, :])

        # ---- mask precompute (partition 0) --------------------------------
        maskf = misc_pool.tile([1, B_LOC, N], F32, tag="mask")
        nc.vector.tensor_copy(maskf[:], mask_i[:])
        mx = const.tile([1, B_LOC, 1], F32)
        nc.vector.reduce_max(mx[:], maskf[:], axis=AX.X)
        suppress = const.tile([1, B_LOC, N], F32)
        nc.vector.tensor_scalar(suppress[:], maskf[:], 1e30, -1e30,
                                op0=OP.mult, op1=OP.add)

        # ---- query^T + tq^T = (query @ Ws)^T as [p(ad), adc, b] -----------
        qT_sb = const.tile([P, NQD, B_LOC], F16)
        for c in range(NQD):
            ps_q = psum_tr.tile([P, B_LOC], F16, tag="tr")
            nc.tensor.transpose(ps_q[:], q_sb[:, c * P:(c + 1) * P],
                                ident_h[:B_LOC, :B_LOC])
            nc.vector.tensor_copy(qT_sb[:, c, :], ps_q[:])
        tqT_sb = const.tile([P, NAD, B_LOC], F32)
        for adc in range(NAD):
            ps_tq = psum_sm.tile([P, B_LOC], F32, tag="sm")
            for qc in range(NQD):
                nc.tensor.matmul(
                    ps_tq[:], Ws_sb[:, qc, adc * P:(adc + 1) * P],
                    qT_sb[:, qc, :], start=(qc == 0), stop=(qc == NQD - 1))
            nc.vector.tensor_copy(tqT_sb[:, adc, :], ps_tq[:])

        # persistent softmax result tile
        w_sb = const.tile([1, B_LOC, N], F32)

        def emit_wT_D_out(b):
            """Transpose w(b), then context matmuls against natural mv(b)."""
            ps_wT = psum_sm.tile([P, NNT], F32, tag="sm")
            for t in range(NNT):
                nc.tensor.matmul(ps_wT[:, t:t + 1],
                                 w_sb[0:1, b, t * P:(t + 1) * P],
                                 ident_f[0:1, 0:1], is_transpose=True,
                                 skip_group_check=True)
            wT = small.tile([P, NNT, 2], F16, tag="wT")
            nc.vector.tensor_copy(wT[:, :, 0:1], ps_wT[:].unsqueeze(-1))
            out_sb = out_pool.tile([1, MD], F32)
            for md2 in range(2):
                ps_c = psum_sm.tile([1, MD // 2], F32, tag="sm")
                for t in range(NNT):
                    nc.tensor.matmul(
                        ps_c[:], wT[:, t, 0:1],
                        naths[b][:, t, md2 * 512:(md2 + 1) * 512],
                        start=(t == 0), stop=(t == NNT - 1),
                        skip_group_check=True)
                nc.vector.tensor_copy(out_sb[0:1, md2 * 512:(md2 + 1) * 512],
                                      ps_c[:])
            nc.sync.dma_start(ctx_d[b:b + 1, :], out_sb[:])

        for b in range(B_LOC):
            if b + 1 < B_LOC:
                emit_loads(b + 1)

            mvT = mvTs[b]
            ps_log = psum_sm.tile([1, N], F32, tag="sm")
            hTs = []
            for adc in range(NAD):
                ps_h = psum_h.tile([P, N], F32)
                for mdc in range(NMD):
                    nc.tensor.matmul(
                        ps_h[:], Wh_sb[:, mdc, adc * P:(adc + 1) * P],
                        mvT[:, mdc, :], start=(mdc == 0),
                        stop=(mdc == NMD - 1))
                hT = hT_pool.tile([P, N], F16)
                nc.scalar.activation(hT[:], ps_h[:], AF.Tanh,
                                     bias=tqT_sb[:, adc, b:b + 1])
                hTs.append(hT)
                if adc >= 1:
                    nc.tensor.matmul(ps_log[:], v_sb[:, adc - 1, 0:1],
                                     hTs[adc - 1], start=(adc - 1 == 0),
                                     stop=False, skip_group_check=True)

            # fill the tanh(7) latency with last batch's context stage
            if b > 0:
                emit_wT_D_out(b - 1)

            nc.tensor.matmul(ps_log[:], v_sb[:, NAD - 1, 0:1], hTs[NAD - 1],
                             start=False, stop=True, skip_group_check=True)

            # masked softmax on partition 0
            ml = small.tile([1, N], F32, tag="ml")
            nc.vector.scalar_tensor_tensor(
                ml[:], in0=suppress[0:1, b, :], scalar=mx[0:1, b, :],
                in1=ps_log[:], op0=OP.mult, op1=OP.add)
            rmx = small.tile([1, 1], F32, tag="rmx")
            nc.vector.reduce_max(rmx[:], ml[:], axis=AX.X)
            nrmx = small.tile([1, 1], F32, tag="nrmx")
            nc.vector.tensor_scalar(nrmx[:], rmx[:], -1.0, None, op0=OP.mult)
            et = small.tile([1, N], F32, tag="et")
            zs = small.tile([1, 1], F32, tag="zs")
            nc.scalar.activation(et[:], ml[:], AF.Exp, bias=nrmx[:],
                                 accum_out=zs[:])
            rz = small.tile([1, 1], F32, tag="rz")
            nc.vector.reciprocal(rz[:], zs[:])
            nc.vector.tensor_scalar(w_sb[0:1, b, :], et[:], rz[:],
                                    mx[0:1, b, :], op0=OP.mult, op1=OP.mult)

        emit_wT_D_out(B_LOC - 1)

    nc.compile()
    return nc


def _get_nc():
    if "nc" not in _CACHE:
        _CACHE["nc"] = _build_nc()
    return _CACHE["nc"]


def kernel(memory_values, mask, query, Wh, Ws, v):
    from concourse.bass_utils import run_bass_kernel_spmd

    nc = _get_nc()
    memory_values = np.ascontiguousarray(memory_values, dtype=np.float32)
    mask = np.ascontiguousarray(mask, dtype=np.int32)
    query = np.ascontiguousarray(query, dtype=np.float32)
    Wh = np.ascontiguousarray(Wh, dtype=np.float32)
    Ws = np.ascontiguousarray(Ws, dtype=np.float32)
    v = np.ascontiguousarray(v, dtype=np.float32)

    in_maps = []
    for c in range(N_CORES):
        s = slice(c * B_LOC, (c + 1) * B_LOC)
        in_maps.append({
            "memory_values": memory_values[s],
            "mask": mask[s],
            "query": query[s],
            "Wh": Wh,
            "Ws": Ws,
            "v": v,
        })
    res = run_bass_kernel_spmd(nc, in_maps, core_ids=list(range(N_CORES)))
    out = np.concatenate([res.results[c]["context"] for c in range(N_CORES)],
                         axis=0)
    return out.astype(np.float32)
